# revision 1
# baseline (speedup 1.0000x reference)
"""Trainium2 Bass kernel for nn_APDTFlow (multi-scale RK4 neural-ODE + GRU).

Sharding: 8 cores = 4 scales x 2 batch halves. Each core:
  - decomposes (conv+BN+relu x2) the FULL batch for its scale (BN needs
    full-batch stats; x is rotated host-side so the core's half is rows 0:128),
  - integrates the RK4 neural ODE for its scale on its half (B=128) using an
    incremental PSUM formulation:
        u1 state (pre-activation of the dynamics MLP's hidden layer) is kept
        as a running PSUM accumulator; each RK4 sub-eval is one fused
        relu(+bias-table) -> small prescaled-weight matmul transition.
        h/lv are recovered at the end from two PSUM accumulators of relu
        outputs (coefficients dt/6 and dt/3).
  - computes per-scale attention score & uncertainty, AllGathers
    (mean, z) across the 4 scale-cores of its half, fuses, and runs the
    48-step GRU decoder on the half.

Host side prepares all derived weights (W_prod = w2mu @ w1h prescaled by RK4
coefficients, bias table with the time feature folded in, transposed GRU
weights) and assembles the full output.
"""
import hashlib
import os
import numpy as np

import concourse.bass as bass
import concourse.bacc as bacc
import concourse.mybir as mybir
from concourse import tile
from concourse.bass_utils import run_bass_kernel_spmd

S, B, T, H, KK, FH, OUT = 4, 256, 256, 128, 5, 48, 1
DW, AW = 64, 32
EPS_BN, EPS_U = 1e-5, 1e-6
BH = 128          # batch half per core
NSTEP = T - 1
NEVAL = 4 * NSTEP
NC = 8
JK = (5, 5, 5, 5)     # junk MMs per ODE stall point (r1..r4)
JGRU = 32             # junk MMs per GRU step
HB2 = 64

f32 = np.float32
F32 = mybir.dt.float32
F16 = mybir.dt.float16

LAST_EXEC_NS = None
_CACHE = {}


# ---------------------------------------------------------------- host prep

def _eval_meta(t_span):
    """Per-step fp32 time math, replicating jax argmin tie-breaking."""
    t = t_span.astype(f32)
    midx_up = []
    times = []            # (t0, tm, t1) per step
    for i in range(NSTEP):
        t0, t1 = t[i], t[i + 1]
        dt = f32(t1 - t0)
        tm = f32(t0 + f32(dt / f32(2.0)))
        im = int(np.argmin(np.abs(t - tm)))
        assert im in (i, i + 1), (i, im)
        midx_up.append(im == i + 1)
        times.append((f32(t0), tm, f32(t1)))
    dtu = f32(t[1] - t[0])
    return midx_up, times, dtu


def _prep(inputs):
    g = lambda k: np.asarray(inputs[k], f32)
    x2d = g("x")[:, 0, :]                      # [B, T]
    t_span = g("t_span")
    midx_up, times, dtu = _eval_meta(t_span)

    dw1, db1 = g("dw1"), g("db1")              # [S,H+2,DW], [S,DW]
    dw2, db2 = g("dw2"), g("db2")              # [S,DW,2H], [S,2H]

    per_dev = []
    for d in range(NC):
        s, gh = d % 4, d // 4
        w1 = dw1[s]
        w1h, w1x, w1t = w1[:H], w1[H], w1[H + 1]
        w2mu, w2lv = dw2[s][:, :H], dw2[s][:, H:]
        b2mu, b2lv = db2[s][:H], db2[s][H:]
        b1 = db1[s]

        Wp = (w2mu @ w1h).astype(f32)          # [DW, DW]
        b2w = (b2mu @ w1h).astype(f32)         # [DW]
        eye = np.eye(DW, dtype=f32)
        Z = np.zeros((DW, DW), f32)
        Wh, Whn, Wf = dtu / 2 * Wp, -dtu / 2 * Wp, dtu * Wp
        W16, W13, W23n = dtu / 6 * Wp, dtu / 3 * Wp, -2 * dtu / 3 * Wp
        # K=128 stacked lhsT blocks, each [128, 128]: rows = (low, high)
        # halves of the paired relu tile; out cols 64:128 zero so the full
        # 128-wide PE array is streamed (HAM activity) but psU[64:] stays 0.
        def blk(lo, hi):
            b = np.zeros((2 * DW, 2 * DW), f32)
            b[:DW, :DW] = lo
            b[DW:, :DW] = hi
            return b
        # PA holds (r1 | r2), PB holds (r3 | r4) in its partition halves.
        blocks = [blk(Wh, Z), blk(Whn, Wh), blk(Wf, Z), blk(Z, Whn),
                  blk(W16, W13), blk(W23n, W16), blk(eye, Z), blk(Z, eye)]
        wtrans = np.concatenate(blocks, axis=1)

        # bias table [DW, NEVAL]
        bt = np.zeros((DW, NEVAL), f32)
        for i in range(NSTEP):
            t0, tm, t1 = times[i]
            Cb = f32(i) * dtu * b2w
            for p, (tt, extra) in enumerate([(t0, 0.0), (tm, dtu / 2),
                                             (tm, dtu / 2), (t1, dtu)]):
                bt[:, 4 * i + p] = b1 + f32(tt) * w1t + Cb + f32(extra) * b2w
        # self-correcting fp16 delta-bias table: the bias rides inside psU
        # (added by rank-1 matmuls) so the relu is a 1-op max. Each delta
        # absorbs the fp16 rounding of all previous ones, so drift never
        # accumulates. Weight APs may only start at partition 0/32/64, so
        # eval e lives at partition 32*(e%3), col block (e//3)*64.
        nbk = (NEVAL + 2) // 3
        btt = np.zeros((128, nbk * 64), np.float16)
        running = np.zeros(DW, np.float64)
        for e in range(NEVAL):
            d = np.float16(bt[:, e].astype(np.float64) - running)
            running += d.astype(np.float64)
            btt[32 * (e % 3), (e // 3) * 64:(e // 3) * 64 + DW] = d

        wfin = np.concatenate([dtu / 6 * w2mu, dtu / 3 * w2mu,
                               dtu / 6 * w2lv, dtu / 3 * w2lv],
                              axis=1).astype(f32)          # [DW, 4H]
        t_end = f32(NSTEP) * dtu
        bfin = np.stack([t_end * b2mu, t_end * b2lv], axis=1).astype(f32)

        cw = np.zeros((128, 2 * KK), f32)
        cw[:, :KK] = g("c1w")[s, 0, 0][None, :]
        cw[:, KK:] = g("c2w")[s, 0, 0][None, :]
        sp = np.zeros((1, 8), f32)
        sp[0, :4] = [g("bn1_g")[s, 0], g("bn1_b")[s, 0],
                     g("bn2_g")[s, 0], g("bn2_b")[s, 0]]

        per_dev.append({
            "xrot": np.roll(x2d, -gh * BH, axis=0).astype(f32),
            "cw": cw,
            "sparams": sp,
            "wtrans": wtrans.astype(np.float16),
            "w1xrow": np.pad(w1x, (0, DW))[None, :].astype(np.float16),
            "btt": btt,
            "wfin": wfin,
            "bfin": bfin,
        })

    whh = g("whh")                              # [3H, H]
    wih = g("wih")[:, 0]                        # [3H]
    fcw = g("fcw")[:, 0]                        # [H]
    tok0v = float(np.asarray(inputs["init_token"]).reshape(-1)[0])
    # gates use tok_t = pred_{t-1} = h_{t-1} @ fcw, same h as the gh matmul,
    # so the token path folds into the gate weights as a rank-1 update:
    #   lhsT_gate = whh_gate^T + fcw (x) wih_gate
    def gw(gate):
        return (whh[gate * H:(gate + 1) * H, :].T
                + np.outer(fcw, wih[gate * H:(gate + 1) * H])).astype(f32)
    whhT = np.concatenate(
        [gw(0), gw(1), whh[2 * H:, :].T,
         np.outer(fcw, wih[2 * H:])], axis=1)   # [H, 4H]: rf, zf, n, gin
    whhT0 = np.concatenate([whh[i * H:(i + 1) * H, :].T for i in range(3)],
                           axis=1)              # [H, 3H] plain (step 0)
    bih, bhh = g("bih"), g("bhh")
    br = bih[:H] + bhh[:H]
    bz = bih[H:2 * H] + bhh[H:2 * H]
    grub = np.stack([br, bz, bih[2 * H:], bhh[2 * H:],
                     br + tok0v * wih[:H], bz + tok0v * wih[H:2 * H],
                     bih[2 * H:] + tok0v * wih[2 * H:],
                     -bz, -(bz + tok0v * wih[H:2 * H])],
                    axis=1).astype(f32)         # [H,9]
    fclv = np.concatenate([g("fcw"), g("lvw")], axis=1).astype(f32)  # [H,2]

    rng = np.random.default_rng(1234)
    shared = {
        "jw": (rng.standard_normal((128, 128)) * 0.05).astype(np.float16),
        "onesq": np.ones((97, 128), np.float16),
        "ident": np.eye(128, dtype=f32),
        "onesr": np.ones((1, 128), f32),
        "onesc": np.ones((128, 1), f32),
        "a1w": g("a1w").astype(np.float16),
        "a1b": g("a1b")[:, None].astype(f32),
        "a2w": g("a2w"),
        "whhT": whhT.astype(np.float16),
        "whhT0": whhT0.astype(np.float16),
        "grub": grub, "fclv": fclv.astype(np.float16),
    }
    meta = {
        "midx_up": tuple(midx_up),
        "a2b": float(g("a2b")[0]),
        "fcb": float(g("fcb")[0]),
        "lvb": float(g("lvb")[0]),
        "tok0": float(np.asarray(inputs["init_token"]).reshape(-1)[0]),
        "has_bhhn": bool(np.any(grub[:, 3])),
    }
    return meta, shared, per_dev


# ---------------------------------------------------------------- graph build

def _build(meta, shared, pd0):
    nc = bacc.Bacc(num_devices=NC)
    tc_args = {}

    # parameters (per-device data via in_maps)
    P = {}
    for name, arr in pd0.items():
        P[name] = nc.declare_dram_parameter(
            name, list(arr.shape), mybir.dt.from_np(arr.dtype), isOutput=False)
    out_ext = nc.declare_dram_parameter("out", [2, FH * BH], F32, isOutput=True)

    C = {k: nc.inline_tensor(v, name=k) for k, v in shared.items()}

    midx_up = meta["midx_up"]
    Relu = mybir.ActivationFunctionType.Relu
    Sig = mybir.ActivationFunctionType.Sigmoid
    Tanh = mybir.ActivationFunctionType.Tanh
    Exp = mybir.ActivationFunctionType.Exp
    Sqrt = mybir.ActivationFunctionType.Sqrt
    Sq = mybir.ActivationFunctionType.Square
    Copy = mybir.ActivationFunctionType.Copy
    Ident = mybir.ActivationFunctionType.Identity
    ADD = mybir.AluOpType.add
    SUB = mybir.AluOpType.subtract
    MUL = mybir.AluOpType.mult

    with tile.TileContext(nc, **tc_args) as tc:
        with (
            tc.tile_pool(name="const", bufs=1) as cp,
            tc.tile_pool(name="work", bufs=2) as wp,
            tc.tile_pool(name="dramp", bufs=1, space="DRAM") as dp,
        ):
            # ---- load params/constants into SBUF
            def load(name, shape, dtype=F32, src=None, tag=None):
                t_ = cp.tile(shape, dtype, name="sb_" + name, tag=tag or name)
                nc.sync.dma_start(t_[:], (src or P[name])[:])
                return t_

            X0 = cp.tile([128, T], F32, name="X0", tag="X0")
            X1 = cp.tile([128, T], F32, name="X1", tag="X1")
            nc.sync.dma_start(X0[:], P["xrot"][0:128, :])
            nc.sync.dma_start(X1[:], P["xrot"][128:256, :])
            cw = load("cw", [128, 2 * KK])
            sparams = load("sparams", [1, 8])
            wtrans = load("wtrans", [2 * DW, 16 * DW], F16)
            w1xrow = load("w1xrow", [1, 2 * DW], F16)
            btt = load("btt", [128, ((NEVAL + 2) // 3) * 64], F16)
            jw = load("jw", [128, 128], F16, C["jw"])
            onesq = load("onesq", [97, BH], F16, C["onesq"])
            wfin = load("wfin", [DW, 4 * H])
            bfin = load("bfin", [H, 2])
            ident = load("ident", [128, 128], F32, C["ident"])
            onesr = load("onesr", [1, 128], F32, C["onesr"])
            onesr16 = cp.tile([1, 128], F16, name="onesr16", tag="onesr16")
            nc.vector.tensor_copy(onesr16[:], onesr[:])
            ident16 = cp.tile([128, 128], F16, name="ident16", tag="ident16")
            nc.vector.tensor_copy(ident16[:], ident[:])
            onesc = load("onesc", [128, 1], F32, C["onesc"])
            a1w = load("a1w", [H, AW], F16, C["a1w"])
            a1b = load("a1b", [AW, 1], F32, C["a1b"])
            a2w = load("a2w", [AW, 1], F32, C["a2w"])
            whhT = load("whhT", [H, 4 * H], F16, C["whhT"])
            whhT0 = load("whhT0", [H, 3 * H], F16, C["whhT0"])
            grub = load("grub", [H, 9], F32, C["grub"])
            fclv = load("fclv", [H, 2], F16, C["fclv"])

            # ================= Phase 1: decomposition + dx staging =========
            with tc.tile_pool(name="psdec", bufs=1, space="PSUM") as pdc:

                def conv_bn_relu(in0, in1, coff, gcol, bcol, relu_both, outtag):
                    ys = []
                    for hi, xin in enumerate((in0, in1)):
                        xp = wp.tile([128, T + 4], F32, name=f"xp{outtag}{hi}",
                                     tag="xpad", bufs=2)
                        nc.vector.memset(xp[:, 0:2], 0.0)
                        nc.vector.memset(xp[:, T + 2:T + 4], 0.0)
                        nc.vector.tensor_copy(xp[:, 2:T + 2], xin[:])
                        y = wp.tile([128, T], F32, name=f"y{outtag}{hi}",
                                    tag=f"y{outtag}", bufs=2)
                        tms = [wp.tile([128, T], F32, name=f"tm{outtag}{hi}{k}",
                                       tag=f"ctmp{k}", bufs=2)
                               for k in range(1, KK)]
                        nc.scalar.activation(y[:], xp[:, 0:T], Copy,
                                             scale=cw[:, coff:coff + 1])
                        for k in range(1, KK):
                            nc.scalar.activation(
                                tms[k - 1][:], xp[:, k:k + T], Copy,
                                scale=cw[:, coff + k:coff + k + 1])
                        for k in range(1, KK):
                            nc.vector.tensor_tensor(y[:], y[:], tms[k - 1][:],
                                                    ADD)
                        ys.append(y)

                    # BN batch stats over both halves
                    scol = [wp.tile([128, 2], F32, name=f"sc{outtag}{hi}",
                                    tag="scol", bufs=2) for hi in range(2)]
                    sqs = wp.tile([128, T], F32, name=f"sq{outtag}",
                                  tag="ctmp", bufs=2)
                    for hi in range(2):
                        nc.vector.tensor_reduce(scol[hi][:, 0:1], ys[hi][:],
                                                mybir.AxisListType.X, ADD)
                        nc.scalar.activation(sqs[:], ys[hi][:], Sq,
                                             accum_out=scol[hi][:, 1:2])
                    pbn = pdc.tile([1, 2], F32, name=f"pbn{outtag}", tag="pbn")
                    nc.tensor.matmul(pbn[:], onesc[:], scol[0][:], start=True,
                                     stop=False, skip_group_check=True)
                    nc.tensor.matmul(pbn[:], onesc[:], scol[1][:], start=False,
                                     stop=True, skip_group_check=True)
                    # scalar math on partition 0
                    invN = 1.0 / (B * T)
                    st = wp.tile([1, 8], F32, name=f"st{outtag}", tag="bnst",
                                 bufs=2)
                    # st cols: m, ex2, v, vpe, sd, y0, a, c
                    nc.scalar.activation(st[:, 0:1], pbn[:, 0:1], Copy,
                                         scale=invN)
                    nc.scalar.activation(st[:, 1:2], pbn[:, 1:2], Copy,
                                         scale=invN)
                    msq = wp.tile([1, 4], F32, name=f"ms{outtag}", tag="bnst2",
                                  bufs=2)
                    nc.scalar.activation(msq[:, 0:1], st[:, 0:1], Sq)
                    nc.vector.tensor_tensor(st[:, 2:3], st[:, 1:2],
                                            msq[:, 0:1], SUB)
                    nc.scalar.activation(st[:, 3:4], st[:, 2:3], Copy,
                                         bias=EPS_BN)
                    nc.scalar.activation(st[:, 4:5], st[:, 3:4], Sqrt)
                    nc.vector.reciprocal(st[:, 5:6], st[:, 4:5])
                    nc.vector.tensor_tensor(st[:, 6:7], st[:, 5:6],
                                            sparams[:, gcol:gcol + 1], MUL)
                    nc.vector.tensor_tensor(msq[:, 3:4], st[:, 6:7],
                                            st[:, 0:1], MUL)
                    nc.vector.tensor_tensor(st[:, 7:8],
                                            sparams[:, bcol:bcol + 1],
                                            msq[:, 3:4], SUB)
                    pab = pdc.tile([128, 2], F32, name=f"pab{outtag}",
                                   tag="pab")
                    nc.tensor.matmul(pab[:], onesr[:], st[:, 6:8], start=True,
                                     stop=True)
                    ab = wp.tile([128, 2], F32, name=f"ab{outtag}", tag="ab",
                                 bufs=2)
                    nc.scalar.copy(ab[:], pab[:])
                    outs = []
                    nh = 2 if relu_both else 1
                    for hi in range(nh):
                        o = wp.tile([128, T], F32, name=f"h{outtag}{hi}",
                                    tag=f"h{outtag}", bufs=2)
                        nc.scalar.activation(o[:], ys[hi][:], Relu,
                                             bias=ab[:, 1:2], scale=ab[:, 0:1])
                        outs.append(o)
                    return outs

                h1 = conv_bn_relu(X0, X1, 0, 0, 1, True, "c1")
                xs = conv_bn_relu(h1[0], h1[1], KK, 2, 3, False, "c2")
                xs0 = xs[0]                   # [128, T] fp32, this core's half

                # ---- dx staging
                dxB = wp.tile([128, T - 1], F32, name="dxB", tag="dxB")
                nc.vector.tensor_tensor(dxB[:], xs0[:, 1:T], xs0[:, 0:T - 1],
                                        SUB)
                DXROW = cp.tile([1, NSTEP * BH], F16, name="DXROW",
                                tag="DXROW")
                for blk in range(2):
                    w = 128 if blk == 0 else NSTEP - 128
                    pt = pdc.tile([w, 128], F32, name=f"pdx{blk}", tag="pdx",
                                  bufs=2)
                    nc.tensor.transpose(pt[:], dxB[:, blk * 128:blk * 128 + w],
                                        ident[:])
                    dxt = wp.tile([w, 128], F16, name=f"dxt{blk}", tag="dxt",
                                  bufs=2)
                    nc.scalar.copy(dxt[:], pt[:])
                    dst = DXROW[0:1, blk * 128 * BH:(blk * 128 + w) * BH]
                    nc.sync.dma_start(dst.rearrange("p (t b) -> p t b", b=BH),
                                      dxt[:])
                px0 = pdc.tile([1, 128], F32, name="px0", tag="px0")
                nc.tensor.matmul(px0[:], xs0[:, 0:1], ident[:], start=True,
                                 stop=True)
                XR0 = cp.tile([1, BH], F16, name="XR0", tag="XR0")
                nc.scalar.copy(XR0[:], px0[:])

            # ================= Phase 2: ODE main loop ======================
            # Single serial chain over the 128-col batch half; relu on DVE.
            # PA holds (r1|r2), PB holds (r3|r4) in partition halves. The
            # eval bias is accumulated INTO psU by rank-1 delta matmuls so
            # the relu is a cheap 1-op max.
            R16sb = wp.tile([DW, BH], F32, name="R16sb", tag="R16sb")
            R13sb = wp.tile([DW, BH], F32, name="R13sb", tag="R13sb")
            PA = cp.tile([2 * DW, BH], F16, name="PA", tag="PA")
            PB = cp.tile([2 * DW, BH], F16, name="PB", tag="PB")
            nc.vector.memset(PA[:], 0.0)
            nc.vector.memset(PB[:], 0.0)
            # PE warm-up: ~6us of full-array dense junk matmuls so HAM
            # un-throttles the PE clock (1.2 -> 2.4 GHz) before the loop.
            with tc.tile_pool(name="pswarm", bufs=1, space="PSUM") as pwarm:
                junk = pwarm.tile([128, BH], F32, name="junk", tag="junk")
                for w_ in range(64):
                    nc.tensor.matmul(junk[:], jw[:], jw[:],
                                     start=True, stop=True,
                                     skip_group_check=True)
            with tc.tile_pool(name="podE", bufs=1, space="PSUM") as podE:
                psU = podE.tile([128, BH], F32, name="psU", tag="psU")
                psR16 = podE.tile([128, BH], F32, name="psR16", tag="psR16")
                psR13 = podE.tile([128, BH], F32, name="psR13", tag="psR13")
                # Junk-fill: HAM re-throttles the PE to 1.2 GHz when array
                # activity drops, and once cold it never re-warms mid-loop.
                # Dep-free full-array dense matmuls race ahead via the PE's
                # 64-deep reorder window and soak up every stall; supply is
                # sized slightly above demand so the window never drains.
                psJ = podE.tile([128, BH], F32, name="psJ", tag="psJ")

                def jfill(n):
                    for _ in range(n):
                        nc.tensor.matmul(psJ[:], jw[:], jw[:],
                                         start=True, stop=True,
                                         skip_group_check=True)

                WT = lambda j: wtrans[:, j * 2 * DW:(j + 1) * 2 * DW]

                def mmp(ps, j, rhs, first=False):
                    nc.tensor.matmul(ps[:], WT(j), rhs, start=first,
                                     stop=False, skip_group_check=True)

                def relu(pt, p0, e):
                    nc.vector.tensor_scalar(pt[p0:p0 + DW, :], psU[0:DW, :],
                                            0.0, None, mybir.AluOpType.max)

                def biasmm(e):
                    # lhsT and rhs must share an SB base partition, and
                    # LDWEIGHTS bases must be 32-aligned
                    r, c = 32 * (e % 3), (e // 3) * DW
                    nc.tensor.matmul(psU[0:DW, :], btt[r:r + 1, c:c + DW],
                                     onesq[r:r + 1, :], start=False,
                                     stop=False, skip_group_check=True)

                nc.tensor.matmul(psU[:], w1xrow[:], XR0[:], start=True,
                                 stop=False, skip_group_check=True)
                biasmm(0)
                first16 = first13 = True
                for i in range(NSTEP):
                    dxap = DXROW[0:1, i * BH:(i + 1) * BH]
                    relu(PA, 0, 4 * i)                   # r1
                    mmp(psU, 0, PA[:])                   # T12
                    if midx_up[i]:
                        nc.tensor.matmul(psU[:], w1xrow[:], dxap, start=False,
                                         stop=False, skip_group_check=True)
                    biasmm(4 * i + 1)
                    jfill(JK[1])
                    relu(PA, DW, 4 * i + 1)              # r2
                    mmp(psU, 1, PA[:])                   # T23
                    biasmm(4 * i + 2)
                    mmp(psR16, 6, PA[:], first=first16)  # R(r1)
                    first16 = False
                    jfill(JK[2])
                    relu(PB, 0, 4 * i + 2)               # r3
                    mmp(psU, 2, PB[:])                   # T34
                    mmp(psU, 3, PA[:])
                    if not midx_up[i]:
                        nc.tensor.matmul(psU[:], w1xrow[:], dxap, start=False,
                                         stop=False, skip_group_check=True)
                    biasmm(4 * i + 3)
                    mmp(psR13, 7, PA[:], first=first13)  # R(r2)
                    first13 = False
                    jfill(JK[3])
                    relu(PB, DW, 4 * i + 3)              # r4
                    if i < NSTEP - 1:                    # T41
                        mmp(psU, 4, PA[:])
                        mmp(psU, 5, PB[:])
                        biasmm(4 * i + 4)
                    mmp(psR13, 6, PB[:])                 # R(r3)
                    mmp(psR16, 7, PB[:])                 # R(r4)
                    jfill(JK[0])

                nc.scalar.copy(R16sb[:], psR16[0:DW, :])
                nc.scalar.copy(R13sb[:], psR13[0:DW, :])

            # ================= Phase 3: recover + attention + fuse ==========
            fusedT = wp.tile([H, BH], F16, name="fusedT", tag="fusedT")
            with tc.tile_pool(name="psfus", bufs=1, space="PSUM") as pf:
                pmean = pf.tile([H, BH], F32, name="pmean", tag="pmean",
                                bufs=2)
                nc.tensor.matmul(pmean[:], wfin[:, 0:H], R16sb[:], start=True,
                                 stop=False, skip_group_check=True)
                nc.tensor.matmul(pmean[:], wfin[:, H:2 * H], R13sb[:],
                                 start=False, stop=True, skip_group_check=True)
                meanT = wp.tile([H, BH], F16, name="meanT", tag="meanT")
                nc.scalar.activation(meanT[:], pmean[:], Ident,
                                     bias=bfin[:, 0:1])
                plv = pf.tile([H, BH], F32, name="plv", tag="pmean", bufs=2)
                nc.tensor.matmul(plv[:], wfin[:, 2 * H:3 * H], R16sb[:],
                                 start=True, stop=False, skip_group_check=True)
                nc.tensor.matmul(plv[:], wfin[:, 3 * H:4 * H], R13sb[:],
                                 start=False, stop=True, skip_group_check=True)
                expvT = wp.tile([H, BH], F32, name="expvT", tag="expvT")
                nc.scalar.activation(expvT[:], plv[:], Exp, bias=bfin[:, 1:2])

                pun = pf.tile([1, BH], F32, name="pun", tag="prow")
                nc.tensor.matmul(pun[:], onesc[:], expvT[:], start=True,
                                 stop=True)
                ps1 = pf.tile([AW, BH], F32, name="ps1", tag="ps1")
                nc.tensor.matmul(ps1[:], a1w[:], meanT[:], start=True,
                                 stop=True)
                rs1 = wp.tile([AW, BH], F32, name="rs1", tag="rs1")
                nc.scalar.activation(rs1[:], ps1[:], Relu, bias=a1b[:, 0:1])
                psc = pf.tile([1, BH], F32, name="psc", tag="prow2")
                nc.tensor.matmul(psc[:], a2w[:], rs1[:], start=True, stop=True)

                denom = wp.tile([1, BH], F32, name="denom", tag="denom")
                nc.scalar.activation(denom[:], pun[:], Copy, scale=1.0 / H,
                                     bias=EPS_U)
                recd = wp.tile([1, BH], F32, name="recd", tag="recd")
                nc.vector.reciprocal(recd[:], denom[:])
                scrow = wp.tile([1, BH], F32, name="scrow", tag="scrow")
                nc.scalar.activation(scrow[:], psc[:], Copy, bias=meta["a2b"])
                zrow = wp.tile([1, BH], F16, name="zrow", tag="zrow")
                nc.vector.tensor_tensor(zrow[:], scrow[:], recd[:], MUL)

                # ---- exchange (mean, z) across the 4 scale-cores of the half
                cc_in = dp.tile([H + 1, BH], F16, name="cc_in", tag="cc_in")
                cc_out = dp.tile([4, H + 1, BH], F16, name="cc_out",
                                 tag="cc_out")
                nc.sync.dma_start(cc_in[0:H, :], meanT[:])
                nc.sync.dma_start(cc_in[H:H + 1, :], zrow[:])
                nc.gpsimd.collective_compute(
                    "AllGather", mybir.AluOpType.bypass,
                    ins=[cc_in.opt()], outs=[cc_out.opt()],
                    replica_groups=[[0, 1, 2, 3], [4, 5, 6, 7]])

                MT = []
                for s in range(4):
                    mt = cp.tile([H, BH], F16, name=f"MT{s}", tag=f"MT{s}")
                    nc.sync.dma_start(mt[:], cc_out[s, 0:H, :])
                    MT.append(mt)
                ZAll = wp.tile([4, BH], F16, name="ZAll", tag="ZAll")
                nc.sync.dma_start(ZAll[:], cc_out[:, H, :])

                # softmax over scales (free-dim layout [BH, 4])
                pzt = pf.tile([BH, 4], F16, name="pzt", tag="pzt", bufs=2)
                nc.tensor.transpose(pzt[:], ZAll[:], ident16[0:4, 0:4])
                zt = wp.tile([BH, 4], F32, name="zt", tag="zt")
                nc.scalar.copy(zt[:], pzt[:])
                mx = wp.tile([BH, 1], F32, name="mx", tag="mx")
                nc.vector.tensor_reduce(mx[:], zt[:], mybir.AxisListType.X,
                                        mybir.AluOpType.max)
                mxn = wp.tile([BH, 1], F32, name="mxn", tag="mxn")
                nc.scalar.activation(mxn[:], mx[:], Copy, scale=-1.0)
                ez = wp.tile([BH, 4], F32, name="ez", tag="ez")
                nc.scalar.activation(ez[:], zt[:], Exp, bias=mxn[:, 0:1])
                sm = wp.tile([BH, 1], F32, name="sm", tag="sm")
                nc.vector.tensor_reduce(sm[:], ez[:], mybir.AxisListType.X,
                                        ADD)
                rc = wp.tile([BH, 1], F32, name="rc", tag="rc")
                nc.vector.reciprocal(rc[:], sm[:])
                wgt = wp.tile([BH, 4], F16, name="wgt", tag="wgt")
                nc.scalar.activation(wgt[:], ez[:], Copy, scale=rc[:, 0:1])
                pwt = pf.tile([4, BH], F16, name="pwt", tag="pzt", bufs=2)
                nc.tensor.transpose(pwt[:], wgt[:], ident16[:])
                wrow4 = wp.tile([4, BH], F16, name="wrow4", tag="wrow4")
                nc.scalar.copy(wrow4[:], pwt[:])
                wrow1 = wp.tile([1, 4 * BH], F16, name="wrow1", tag="wrow1")
                nc.sync.dma_start(
                    wrow1[0:1, :].rearrange("p (s b) -> p s b", b=BH),
                    wrow4[:])

                tmpf = wp.tile([H, BH], F16, name="tmpf", tag="tmpf")
                for s in range(4):
                    pwb = pf.tile([H, BH], F32, name=f"pwb{s}", tag="pwb")
                    nc.tensor.matmul(pwb[:], onesr16[:],
                                     wrow1[0:1, s * BH:(s + 1) * BH],
                                     start=True, stop=True)
                    dst = fusedT if s == 0 else tmpf
                    nc.vector.tensor_tensor(dst[:], MT[s][:], pwb[:], MUL)
                    if s > 0:
                        nc.vector.tensor_tensor(fusedT[:], fusedT[:], tmpf[:],
                                                ADD)

            # ================= Phase 4: GRU decoder ========================
            Pout = cp.tile([1, FH * BH], F32, name="Pout", tag="Pout")
            Lout = cp.tile([1, FH * BH], F32, name="Lout", tag="Lout")
            with tc.tile_pool(name="psgru", bufs=1, space="PSUM") as pg:
                psJg = pg.tile([128, BH], F32, name="psJg", tag="psJg")

                def jfillg(n):
                    for _ in range(n):
                        nc.tensor.matmul(psJg[:], jw[:], jw[:],
                                         start=True, stop=True,
                                         skip_group_check=True)

                def out_mms(hsrc, t):
                    # pred/logvar matmuls for output index t (hsrc = h_t);
                    # one 2-row PSUM tile (rows 0/32) so both fit in 2 banks
                    po = pg.tile([33, BH], F32, name=f"po{t}", tag="pout",
                                 bufs=2)
                    nc.tensor.matmul(po[0:1, :], fclv[:, 0:1], hsrc[:],
                                     start=True, stop=True,
                                     skip_group_check=True)
                    nc.tensor.matmul(po[32:33, :], fclv[:, 1:2], hsrc[:],
                                     start=True, stop=True,
                                     skip_group_check=True)
                    return po

                def out_writes(po, t):
                    nc.scalar.activation(Pout[0:1, t * BH:(t + 1) * BH],
                                         po[0:1, :], Copy, bias=meta["fcb"])
                    nc.vector.tensor_scalar(Lout[0:1, t * BH:(t + 1) * BH],
                                            po[32:33, :], meta["lvb"], None,
                                            ADD)

                hT = fusedT
                pend = None          # (pp_, pq_, t-1) awaiting engine writes
                for t in range(FH):
                    pr = pg.tile([H, BH], F32, name=f"pr{t}", tag="pr")
                    pz = pg.tile([H, BH], F32, name=f"pz{t}", tag="pz")
                    pn = pg.tile([H, BH], F32, name=f"pn{t}", tag="pn")
                    if t == 0:
                        nc.tensor.matmul(pr[:], whhT0[:, 0:H], hT[:],
                                         start=True, stop=True,
                                         skip_group_check=True)
                        nc.tensor.matmul(pz[:], whhT0[:, H:2 * H], hT[:],
                                         start=True, stop=True,
                                         skip_group_check=True)
                        nc.tensor.matmul(pn[:], whhT0[:, 2 * H:3 * H], hT[:],
                                         start=True, stop=True,
                                         skip_group_check=True)
                        brc, bzc, bnc = 4, 5, 6
                        pgin = None
                    else:
                        nc.tensor.matmul(pr[:], whhT[:, 0:H], hT[:],
                                         start=True, stop=True,
                                         skip_group_check=True)
                        nc.tensor.matmul(pz[:], whhT[:, H:2 * H], hT[:],
                                         start=True, stop=True,
                                         skip_group_check=True)
                        nc.tensor.matmul(pn[:], whhT[:, 2 * H:3 * H], hT[:],
                                         start=True, stop=True,
                                         skip_group_check=True)
                        pgin = pg.tile([H, BH], F32, name=f"pgin{t}",
                                       tag="pgin")
                        nc.tensor.matmul(pgin[:], whhT[:, 3 * H:4 * H], hT[:],
                                         start=True, stop=True,
                                         skip_group_check=True)
                        brc, bzc, bnc = 0, 1, 2
                    if t > 0:
                        # pred/lv for step t-1 share this step's gate rhs
                        po = out_mms(hT, t - 1)
                    # ACT: reset gate first (it gates the critical n-path)
                    rg = wp.tile([H, BH], F16, name=f"rg{t}", tag="rg", bufs=2)
                    zg = wp.tile([H, BH], F16, name=f"zg{t}", tag="zg", bufs=2)
                    nc.scalar.activation(rg[:], pr[:], Sig,
                                         bias=grub[:, brc:brc + 1])
                    nc.scalar.activation(zg[:], pz[:], Sig,
                                         bias=grub[:, bzc:bzc + 1])
                    t1 = wp.tile([H, BH], F16, name=f"t1_{t}", tag="t1",
                                 bufs=2)
                    if meta["has_bhhn"]:
                        hn = wp.tile([H, BH], F16, name=f"hn{t}", tag="hn",
                                     bufs=2)
                        nc.scalar.activation(hn[:], pn[:], Ident,
                                             bias=grub[:, 3:4])
                        nc.vector.tensor_tensor(t1[:], rg[:], hn[:], MUL)
                    else:
                        nc.vector.tensor_tensor(t1[:], rg[:], pn[:], MUL)
                    if pgin is not None:
                        t2 = wp.tile([H, BH], F16, name=f"t2_{t}", tag="t2",
                                     bufs=2)
                        nc.vector.tensor_tensor(t2[:], t1[:], pgin[:], ADD)
                    else:
                        t2 = t1
                    ng = wp.tile([H, BH], F16, name=f"ng{t}", tag="ng", bufs=2)
                    nc.scalar.activation(ng[:], t2[:], Tanh,
                                         bias=grub[:, bnc:bnc + 1])
                    if pend is not None:     # prev step's Pout ACT write sits
                        out_writes(*pend)    # behind ng, off the chain
                        pend = None
                    # DVE (while ACT runs ng): zcm = zg-1, zh = zg*hT
                    zcm = wp.tile([H, BH], F16, name=f"zc{t}", tag="zc",
                                  bufs=2)
                    nc.vector.tensor_scalar(zcm[:], zg[:], 1.0, None, SUB)
                    zh = wp.tile([H, BH], F16, name=f"zh{t}", tag="zh", bufs=2)
                    nc.vector.tensor_tensor(zh[:], zg[:], hT[:], MUL)
                    # h' = zh - (zg-1)*ng  ==  (1-zg)*ng + zg*h
                    t3 = wp.tile([H, BH], F16, name=f"t3_{t}", tag="t3",
                                 bufs=2)
                    nc.vector.tensor_tensor(t3[:], zcm[:], ng[:], MUL)
                    hT2 = wp.tile([H, BH], F16, name=f"hT{t}", tag="hT",
                                  bufs=3)
                    nc.vector.tensor_tensor(hT2[:], zh[:], t3[:], SUB)
                    hT = hT2
                    if t > 0:
                        pend = (po, t - 1)
                    jfillg(JGRU)
                # tail: final step's outputs
                po = out_mms(hT, FH - 1)
                out_writes(*pend)
                out_writes(po, FH - 1)

            nc.sync.dma_start(out_ext[0:1, :], Pout[:])
            nc.sync.dma_start(out_ext[1:2, :], Lout[:])

    nc.compile()
    return nc


# ---------------------------------------------------------------- entry point

def kernel(**inputs):
    global LAST_EXEC_NS
    meta, shared, per_dev = _prep(inputs)

    hh = hashlib.sha1()
    for k in sorted(inputs):
        if k == "x":
            continue
        hh.update(k.encode())
        hh.update(np.ascontiguousarray(np.asarray(inputs[k])).tobytes())
    key = hh.hexdigest()
    if key not in _CACHE:
        _CACHE[key] = _build(meta, shared, per_dev[0])
    nc = _CACHE[key]

    trace = bool(os.environ.get("BASS_TRACE"))
    res = run_bass_kernel_spmd(nc, per_dev, core_ids=list(range(NC)),
                               trace=trace)
    LAST_EXEC_NS = res.exec_time_ns

    outs = np.zeros((B, FH, OUT), f32)
    lvs = np.zeros((B, FH, OUT), f32)
    for gh in range(2):
        r = res.results[4 * gh]["out"]          # [2, FH*BH]
        outs[gh * BH:(gh + 1) * BH, :, 0] = r[0].reshape(FH, BH).T
        lvs[gh * BH:(gh + 1) * BH, :, 0] = r[1].reshape(FH, BH).T
    return outs, lvs

